# revision 20
# baseline (speedup 1.0000x reference)
"""Trainium2 Bass kernel for the periodic 9-point diffusion stencil.

Full input:  state [4, 8, 1024, 1024] f32, diffusion_coefficient, dt (scalars).
Full output: [4, 8, 1024, 1024] f32.

Math:  out = c2 * X + c1 * (Sv @ Sh(X))   with Sv = Sh = [1,2,1] periodic,
       c1 = scale/12, c2 = 1 - 4*scale/3, scale = dc*dt.
This equals the reference 9-point kernel (center 1-scale, edges scale/6,
corners scale/12); the reference's mass-conservation correction is orders of
magnitude below f32 resolution and enters here only through exact math.

Sharding: 32 independent (b, c) slices of [1024, 1024]; 4 slices per core.
Per slice: 9 overlapping row-tiles (128 rows in -> 126 valid out rows, last
tile K=32) so the vertical wrap needs no cross-partition traffic. The
vertical [1,2,1] filter is a tridiagonal-matrix matmul on the TensorEngine;
horizontal taps are column-shifted rhs views accumulated in PSUM. The PE
operands are bf16: weights are real bf16 tensors, and X is read through a
bitcast high-half view of the f32 tile (truncated bf16). That precision only
touches the ~1e-3-scaled neighbor terms; the dominant identity term c2*X is
computed in exact f32 by one fused VectorEngine scalar_tensor_tensor op that
also reads PSUM and writes the output tile.
"""

import numpy as np

N_CORES = 8
S_PER_CORE = 4  # (b,c) slices per core
H = W = 1024
ROWS_PER_TILE = 126  # valid output rows per full tile
N_FULL_TILES = 8     # 8*126 = 1008 rows; last 16 rows via a K=32 tile
LAST_ROWS = H - N_FULL_TILES * ROWS_PER_TILE  # 16

_PROGRAM = None


def _build_program():
    from contextlib import ExitStack

    import concourse.bass as bass
    import concourse.tile as tile
    from concourse import bacc, mybir
    from concourse.bass_interp import get_hw_module

    f32 = mybir.dt.float32
    bf16 = mybir.dt.bfloat16
    mult = mybir.AluOpType.mult
    add = mybir.AluOpType.add

    nc = bacc.Bacc("TRN2", target_bir_lowering=False, debug=False,
                   num_devices=N_CORES)
    x = nc.dram_tensor("x", [S_PER_CORE, H, W], f32, kind="ExternalInput").ap()
    w1 = nc.dram_tensor("w1", [128, 128], bf16, kind="ExternalInput").ap()
    w2 = nc.dram_tensor("w2", [128, 128], bf16, kind="ExternalInput").ap()
    c2v = nc.dram_tensor("c2v", [128, 1], f32, kind="ExternalInput").ap()
    y = nc.dram_tensor("y", [S_PER_CORE, H, W], f32, kind="ExternalOutput").ap()

    with tile.TileContext(nc) as tc:
        with ExitStack() as ctx:
            consts = ctx.enter_context(tc.tile_pool(name="consts", bufs=1))
            xp = ctx.enter_context(tc.tile_pool(name="x", bufs=4))
            op = ctx.enter_context(tc.tile_pool(name="o", bufs=4))
            pp = ctx.enter_context(tc.tile_pool(name="ps", bufs=3, space="PSUM"))

            w1t = consts.tile([128, 128], bf16)
            nc.sync.dma_start(w1t[:], w1[:])
            w2t = consts.tile([128, 128], bf16)
            nc.sync.dma_start(w2t[:], w2[:])
            c2t = consts.tile([128, 1], f32)
            nc.sync.dma_start(c2t[:], c2v[:])

            def stencil_tile(xt, pt, K):
                """Accumulate the 9-point neighbor sum (scaled by c1) into
                psum tile pt from halo SBUF tile xt ([K, W+2] f32; halo col 0
                = X col 1023, halo col W+1 = X col 0). Matmul rhs is the
                bf16 high-half view of xt; each chunk is a full 512-wide
                matmul with the horizontal shift expressed via the rhs."""
                l1 = w1t[:K, :K]
                l2 = w2t[:K, :K]
                # wrap columns (within-tile copies)
                nc.vector.tensor_copy(xt[:, 0:1], xt[:, 1024:1025])
                nc.vector.tensor_copy(xt[:, 1025:1026], xt[:, 1:2])
                # bf16 high-half view: f32 [K, 1026] -> bf16 [K, 2052],
                # odd elements are the f32 high halves
                xb = xt[:].bitcast(bf16)[:, 1::2]
                for c in (0, 512):
                    # center taps (weight 2*c1*T), first writer of the bank
                    nc.tensor.matmul(pt[:, c:c + 512], l2, xb[:, c + 1:c + 513],
                                     start=True, stop=False,
                                     skip_group_check=True)
                    # left neighbors: psum[:, j] += W1 @ X[:, j-1]
                    nc.tensor.matmul(pt[:, c:c + 512], l1, xb[:, c:c + 512],
                                     start=False, stop=False,
                                     skip_group_check=True)
                    # right neighbors: psum[:, j] += W1 @ X[:, j+1]
                    nc.tensor.matmul(pt[:, c:c + 512], l1, xb[:, c + 2:c + 514],
                                     start=False, stop=True,
                                     skip_group_check=True)

            for s in range(S_PER_CORE):
                for t in range(N_FULL_TILES):
                    r0 = t * ROWS_PER_TILE  # first output row of this tile
                    xt = xp.tile([128, W + 2], f32, tag="xt")
                    if t == 0:
                        # rows -1..126 (row -1 wraps to 1023)
                        nc.sync.dma_start(xt[0:1, 1:1025], x[s, H - 1:H, :])
                        nc.sync.dma_start(xt[1:128, 1:1025], x[s, 0:127, :])
                    else:
                        nc.sync.dma_start(xt[:, 1:1025], x[s, r0 - 1:r0 + 127, :])
                    pt = pp.tile([128, W], f32, tag="pt")
                    stencil_tile(xt, pt, 128)
                    ot = op.tile([128, W], f32, tag="ot")
                    nc.vector.scalar_tensor_tensor(
                        ot[:], xt[:, 1:1025], c2t[:], pt[:],
                        op0=mult, op1=add)
                    nc.sync.dma_start(y[s, r0:r0 + ROWS_PER_TILE, :],
                                      ot[1:1 + ROWS_PER_TILE, :])

                # last LAST_ROWS rows via a K=32 tile:
                # partitions 0..16 = x rows 1007..1023, 17..31 = x rows 0..14
                r0 = N_FULL_TILES * ROWS_PER_TILE  # 1008
                xt = xp.tile([32, W + 2], f32, tag="xt")
                nc.sync.dma_start(xt[0:17, 1:1025], x[s, r0 - 1:H, :])
                nc.sync.dma_start(xt[17:32, 1:1025], x[s, 0:15, :])
                pt = pp.tile([32, W], f32, tag="pt")
                stencil_tile(xt, pt, 32)
                ot = op.tile([32, W], f32, tag="ot")
                nc.vector.scalar_tensor_tensor(
                    ot[:], xt[:, 1:1025], c2t[0:32, :], pt[:],
                    op0=mult, op1=add)
                nc.sync.dma_start(y[s, r0:H, :], ot[1:1 + LAST_ROWS, :])

    nc.compile()
    nc.m = get_hw_module(nc.m)
    return nc


def _get_program():
    global _PROGRAM
    if _PROGRAM is None:
        _PROGRAM = _build_program()
    return _PROGRAM


def kernel(state, diffusion_coefficient, dt):
    import ml_dtypes
    from concourse.bass_utils import run_bass_kernel_spmd

    state = np.asarray(state)
    in_dtype = state.dtype
    xs = np.ascontiguousarray(state, dtype=np.float32).reshape(32, H, W)

    scale = float(np.asarray(diffusion_coefficient, dtype=np.float64)) * \
        float(np.asarray(dt, dtype=np.float64))
    c1 = scale / 12.0
    c2 = 1.0 - 4.0 * scale / 3.0

    tri = np.zeros((128, 128), dtype=np.float64)
    idx = np.arange(128)
    tri[idx, idx] = 2.0
    tri[idx[:-1], idx[:-1] + 1] = 1.0
    tri[idx[:-1] + 1, idx[:-1]] = 1.0
    w1 = (c1 * tri).astype(ml_dtypes.bfloat16)
    w2 = (2.0 * c1 * tri).astype(ml_dtypes.bfloat16)
    c2v = np.full((128, 1), c2, dtype=np.float32)

    nc = _get_program()
    in_maps = [
        {"x": xs[k * S_PER_CORE:(k + 1) * S_PER_CORE], "w1": w1, "w2": w2,
         "c2v": c2v}
        for k in range(N_CORES)
    ]
    res = run_bass_kernel_spmd(nc, in_maps, core_ids=list(range(N_CORES)))
    out = np.concatenate([res.results[k]["y"] for k in range(N_CORES)], axis=0)
    return out.reshape(4, 8, H, W).astype(in_dtype, copy=False)


# revision 32
# speedup vs baseline: 285.5868x; 285.5868x over previous
"""Trainium2 Bass kernel for the periodic 9-point diffusion stencil.

Full input:  state [4, 8, 1024, 1024] f32, diffusion_coefficient, dt (scalars).
Full output: [4, 8, 1024, 1024] f32.

Math:  out = c2 * X + c1 * (Sv @ Sh(X))   with Sv = Sh = [1,2,1] periodic,
       c1 = scale/12, c2 = 1 - 4*scale/3, scale = dc*dt.
This equals the reference 9-point kernel (center 1-scale, edges scale/6,
corners scale/12); the reference's mass-conservation correction is orders of
magnitude below f32 resolution and enters here only through exact math.

Sharding: 32 independent (b, c) slices of [1024, 1024]; 4 slices per core.
Per slice: 9 overlapping row-tiles (128 rows in -> 126 valid out rows, last
tile K=32) so the vertical wrap needs no cross-partition traffic. The
vertical [1,2,1] filter is a tridiagonal-matrix matmul on the TensorEngine;
horizontal taps are column-shifted rhs views accumulated in PSUM, with the
column wrap handled by two extra 1-wide matmuls (tiles stay 1024-wide and
4KB-aligned - a misaligned halo layout costs ~2x DMA bandwidth). PE operands
are bf16: weights are real bf16 tensors and X is read through a bitcast
high-half view of the f32 tile (truncated bf16). That precision only touches
the ~1e-3-scaled neighbor terms; the dominant identity term c2*X is computed
in exact f32 by one fused VectorEngine scalar_tensor_tensor op that also
reads PSUM and writes the output tile.
"""

import numpy as np

N_CORES = 8
S_PER_CORE = 4  # (b,c) slices per core
H = W = 1024
ROWS_PER_TILE = 126  # valid output rows per full tile
N_FULL_TILES = 8     # 8*126 = 1008 rows; last 16 rows via a K=32 tile
LAST_ROWS = H - N_FULL_TILES * ROWS_PER_TILE  # 16

_PROGRAM = None


def _build_program(loop_r=1, x_bufs=3, o_bufs=3, ps_bufs=4, split_combine=0,
                   skip_tail=0, variant="full"):
    from contextlib import ExitStack

    import concourse.bass as bass
    import concourse.tile as tile
    from concourse import bacc, mybir
    from concourse.bass_interp import get_hw_module

    f32 = mybir.dt.float32
    bf16 = mybir.dt.bfloat16
    mult = mybir.AluOpType.mult
    add = mybir.AluOpType.add

    nc = bacc.Bacc("TRN2", target_bir_lowering=False, debug=False,
                   num_devices=N_CORES)
    x = nc.dram_tensor("x", [S_PER_CORE, H, W], f32, kind="ExternalInput").ap()
    w1 = nc.dram_tensor("w1", [128, 128], bf16, kind="ExternalInput").ap()
    w2 = nc.dram_tensor("w2", [128, 128], bf16, kind="ExternalInput").ap()
    c2v = nc.dram_tensor("c2v", [128, 1], f32, kind="ExternalInput").ap()
    y = nc.dram_tensor("y", [S_PER_CORE, H, W], f32, kind="ExternalOutput").ap()

    with tile.TileContext(nc) as tc:
        with ExitStack() as ctx:
            consts = ctx.enter_context(tc.tile_pool(name="consts", bufs=1))
            xp = ctx.enter_context(tc.tile_pool(name="x", bufs=x_bufs))
            op = ctx.enter_context(tc.tile_pool(name="o", bufs=o_bufs))
            pp = ctx.enter_context(
                tc.tile_pool(name="ps", bufs=ps_bufs, space="PSUM"))

            w1t = consts.tile([128, 128], bf16)
            nc.sync.dma_start(w1t[:], w1[:])
            w2t = consts.tile([128, 128], bf16)
            nc.sync.dma_start(w2t[:], w2[:])
            c2t = consts.tile([128, 1], f32)
            nc.sync.dma_start(c2t[:], c2v[:])

            def stencil_tile(xb, pt, K, base=0):
                """Accumulate the 9-point neighbor sum (scaled by c1) into
                psum tile pt. xb is the bf16 high-half view of the f32 SBUF
                data; `base` selects a 1024-wide block within it. Horizontal
                shifts live in the rhs windows, with the two wrap columns
                via 1-wide matmuls."""
                l1 = w1t[:K, :K]
                l2 = w2t[:K, :K]
                b = base
                # center taps (weight 2*c1*T), first writers of both banks
                nc.tensor.matmul(pt[:, 0:512], l2, xb[:, b:b + 512],
                                 start=True, stop=False, skip_group_check=True)
                nc.tensor.matmul(pt[:, 512:1024], l2, xb[:, b + 512:b + 1024],
                                 start=True, stop=False, skip_group_check=True)
                # left neighbors: psum[:, j] += W1 @ X[:, j-1]
                nc.tensor.matmul(pt[:, 1:512], l1, xb[:, b:b + 511],
                                 start=False, stop=False, skip_group_check=True)
                nc.tensor.matmul(pt[:, 0:1], l1, xb[:, b + 1023:b + 1024],
                                 start=False, stop=False, skip_group_check=True)
                nc.tensor.matmul(pt[:, 512:1024], l1, xb[:, b + 511:b + 1023],
                                 start=False, stop=False, skip_group_check=True)
                # right neighbors: psum[:, j] += W1 @ X[:, j+1]
                nc.tensor.matmul(pt[:, 0:512], l1, xb[:, b + 1:b + 513],
                                 start=False, stop=True, skip_group_check=True)
                nc.tensor.matmul(pt[:, 512:1023], l1, xb[:, b + 513:b + 1024],
                                 start=False, stop=False, skip_group_check=True)
                nc.tensor.matmul(pt[:, 1023:1024], l1, xb[:, b:b + 1],
                                 start=False, stop=True, skip_group_check=True)

            def body(_i=None):
                from concourse.ap import AP as mkAP
                for s in range(S_PER_CORE):
                    # super-tiles of 4 row-blocks; block n = x rows
                    # 126n..126n+127 (partition-aligned, no wrap) -> out rows
                    # 126n+1..126n+126. One 2MB multi-dim DMA each way per
                    # super-tile: per-DMA issue cost (~1.7us/queue) dominates
                    # at 512KB granularity.
                    xs_ = x[s]
                    ys_ = y[s]
                    for g in range(2):
                        n0 = 4 * g
                        in_view = mkAP(
                            tensor=xs_.tensor,
                            offset=xs_.offset + 126 * n0 * W,
                            ap=[[W, 128], [126 * W, 4], [1, W]])
                        xt = xp.tile([128, 4 * W], f32, tag="xt")
                        nc.sync.dma_start(
                            xt[:].rearrange("p (n w) -> p n w", n=4), in_view)
                        xb = xt[:].bitcast(bf16)[:, 1::2]
                        ot = op.tile([128, 4 * W], f32, tag="ot")
                        for b in range(4):
                            if variant == "dma":
                                continue
                            pt = pp.tile([128, W], f32, tag="pt")
                            if variant in ("pe", "full"):
                                stencil_tile(xb, pt, 128, base=b * W)
                            if variant == "pe":
                                dot = op.tile([1, 2], f32, tag="dummy")
                                nc.vector.tensor_copy(dot[:], pt[0:1, 0:2])
                                continue
                            src_ = pt[:] if variant == "full" else \
                                xt[:, b * W:(b + 1) * W]
                            eng = nc.gpsimd if (split_combine and b % 2) else \
                                nc.vector
                            eng.scalar_tensor_tensor(
                                ot[:, b * W:(b + 1) * W],
                                xt[:, b * W:(b + 1) * W],
                                c2t[:], src_, op0=mult, op1=add)
                        if variant in ("dma", "pe"):
                            nc.vector.tensor_copy(ot[0:1, 0:2],
                                                  xt[0:1, 0:2])
                        # out-DMAs issue from ACT's HWDGE queue so their
                        # wait-on-combine doesn't stall the sync sequencer's
                        # in-DMA issue stream
                        out_view = mkAP(
                            tensor=ys_.tensor,
                            offset=ys_.offset + (126 * n0 + 1) * W,
                            ap=[[W, 126], [126 * W, 4], [1, W]])
                        nc.scalar.dma_start(
                            out_view,
                            ot[1:127, :].rearrange("p (n w) -> p n w", n=4))

                    # K=32 wrap tile: partitions 0..15 = x rows 1008..1023,
                    # partitions 16..31 = x rows 0..15. Valid psum rows 1..30;
                    # rows 1..15 -> out rows 1009..1023, row 16 -> out row 0
                    # (its taps at partitions 15,16,17 = x rows 1023,0,1 are
                    # exactly the vertical wrap).
                    if skip_tail:
                        continue
                    r0 = N_FULL_TILES * ROWS_PER_TILE + 1  # 1009
                    xt = xp.tile([32, W], f32, tag="xt_last")
                    nc.sync.dma_start(xt[0:16, :], x[s, H - 16:H, :])
                    nc.sync.dma_start(xt[16:32, :], x[s, 0:16, :])
                    pt = pp.tile([32, W], f32, tag="pt")
                    stencil_tile(xt[:].bitcast(bf16)[:, 1::2], pt, 32)
                    ot = op.tile([32, W], f32, tag="ot")
                    nc.vector.scalar_tensor_tensor(
                        ot[:], xt[:], c2t[0:32, :], pt[:], op0=mult, op1=add)
                    nc.scalar.dma_start(y[s, r0:H, :], ot[1:1 + H - r0, :])
                    nc.scalar.dma_start(y[s, 0:1, :], ot[16:17, :])

            if loop_r == 1:
                body()
            else:
                with tc.For_i(0, loop_r, 1):
                    body()

    nc.compile()
    nc.m = get_hw_module(nc.m)
    return nc


def _get_program():
    global _PROGRAM
    if _PROGRAM is None:
        _PROGRAM = _build_program()
    return _PROGRAM


def kernel(state, diffusion_coefficient, dt):
    import ml_dtypes
    from concourse.bass_utils import run_bass_kernel_spmd

    state = np.asarray(state)
    in_dtype = state.dtype
    xs = np.ascontiguousarray(state, dtype=np.float32).reshape(32, H, W)

    scale = float(np.asarray(diffusion_coefficient, dtype=np.float64)) * \
        float(np.asarray(dt, dtype=np.float64))
    c1 = scale / 12.0
    c2 = 1.0 - 4.0 * scale / 3.0

    tri = np.zeros((128, 128), dtype=np.float64)
    idx = np.arange(128)
    tri[idx, idx] = 2.0
    tri[idx[:-1], idx[:-1] + 1] = 1.0
    tri[idx[:-1] + 1, idx[:-1]] = 1.0
    w1 = (c1 * tri).astype(ml_dtypes.bfloat16)
    w2 = (2.0 * c1 * tri).astype(ml_dtypes.bfloat16)
    c2v = np.full((128, 1), c2, dtype=np.float32)

    nc = _get_program()
    in_maps = [
        {"x": xs[k * S_PER_CORE:(k + 1) * S_PER_CORE], "w1": w1, "w2": w2,
         "c2v": c2v}
        for k in range(N_CORES)
    ]
    res = run_bass_kernel_spmd(nc, in_maps, core_ids=list(range(N_CORES)))
    out = np.concatenate([res.results[k]["y"] for k in range(N_CORES)], axis=0)
    return out.reshape(4, 8, H, W).astype(in_dtype, copy=False)
